# revision 1
# baseline (speedup 1.0000x reference)
"""FDN reverb kernel for 8x TRN2 NeuronCores.

Computes out = y / max|y| with y[t] = x[t] + sum_n a_n * x[t - d_n],
where a_n = (sum_j Q[j, n]) * g[n]  (the MIX=0.5 factor cancels in the
normalization).

Sharding: time axis split into 8 contiguous shards of 1M samples; each
core's input carries a max-delay halo from the previous shard (zeros for
core 0).  On-core layout is partition-major: partition p holds samples
[p*F, p*F + F) of the shard plus a D-sample halo in front, so every
delayed read is a free-axis offset.

Delay taps run on the tensor engine as diagonal-stationary matmuls
accumulating in PSUM.  For full fp32-level accuracy each operand is
split into a bf16 hi/lo pair (x = xh + xl, a = ah + al) and three exact
bf16 matmuls per tap compute ah*xh + ah*xl + al*xh (the dropped al*xl
term is ~2^-18 relative).  The identity tap is added exactly on the DVE
while evacuating PSUM.  A tiny AllGather(max) across the 8 cores yields
the global normalizer; DVE/ACT scale; DMA out.

Set PRECISE=False for a single-pass float32r version (~2x faster PE
phase, ~1.8e-4 max relative error from the PE's ~11-bit rounding).
"""

import numpy as np
import ml_dtypes

import concourse.bacc as bacc
import concourse.bass as bass
import concourse.mybir as mybir
import concourse.tile as tile
from concourse.bass_utils import run_bass_kernel_spmd

# ---- problem constants (hardcoded; must match the reference) ----
SAMPLE_RATE = 48000
DELAYS_SEC = [0.0297, 0.0371, 0.0411, 0.0437, 0.0533, 0.0617, 0.0731, 0.0797]
DELAYS = [int(d * SAMPLE_RATE) for d in DELAYS_SEC]  # [1425,...,3825]
NTAPS = len(DELAYS)  # 8
T = 8388608
N_CORES = 8
T_CORE = T // N_CORES  # 1048576
P = 128
F = T_CORE // P  # 8192 samples per partition row
D = 3840  # halo (>= max delay 3825), kept 128-aligned
TILE = 512  # matmul moving free dim / PSUM bank size (fp32)
NTILES = F // TILE  # 16

PRECISE = True

_cache = {}


def _build_nc():
    fp32 = mybir.dt.float32
    bf16 = mybir.dt.bfloat16
    f32r = mybir.dt.float32r
    xdt = bf16 if PRECISE else f32r

    nc = bacc.Bacc(
        "TRN2",
        target_bir_lowering=False,
        debug=False,
        enable_asserts=False,
        num_devices=N_CORES,
    )

    # inputs: hi/lo bf16 streams (or one f32r stream), stationary diagonals
    xh_d = nc.dram_tensor("xh", [1, D + T_CORE], xdt, kind="ExternalInput")
    if PRECISE:
        xl_d = nc.dram_tensor("xl", [1, D + T_CORE], xdt, kind="ExternalInput")
    dsets = 2 if PRECISE else 1  # diag value sets: a_hi, a_lo
    diags = nc.dram_tensor("diags", [P, dsets * NTAPS * P], xdt, kind="ExternalInput")
    if PRECISE:
        # full-fp32 a_lo values for the taps whose al*xh pass runs on DVE
        alv = nc.dram_tensor("alv", [P, NTAPS], fp32, kind="ExternalInput")
    ident = nc.dram_tensor("ident", [P, P], fp32, kind="ExternalInput")
    out = nc.dram_tensor("out", [1, T_CORE], fp32, kind="ExternalOutput")

    def shard_ap(t, c0, c1):
        # columns [c0, c1) of the overlapped [128, D+F] row view
        return bass.AP(tensor=t, offset=c0, ap=[[F, P], [1, c1 - c0]])

    # DMA-in column chunks: small leading chunks so the PE can start early
    bounds = [0, 640, 1664, 2944, 4480, 6016, 8000, 10016, 12032]
    XCH = list(zip(bounds[:-1], bounds[1:]))

    with tile.TileContext(nc) as tc:
        with (
            tc.tile_pool(name="xpool", bufs=1) as xpool,
            tc.tile_pool(name="ypool", bufs=1) as ypool,
            tc.tile_pool(name="dpool", bufs=1) as dpool,
            tc.tile_pool(name="spool", bufs=1) as spool,
            tc.tile_pool(name="psum", bufs=7, space="PSUM") as psum_pool,
            tc.tile_pool(name="psumt", bufs=1, space="PSUM") as psumt_pool,
            tc.tile_pool(name="dram", bufs=1, space="DRAM") as dram_pool,
        ):
            xh_t = xpool.tile([P, D + F], xdt)
            xl_t = xpool.tile([P, D + F], xdt, name="xl_t") if PRECISE else None
            y_tile = ypool.tile([P, F], fp32)
            diag_t = dpool.tile([P, dsets * NTAPS * P], xdt)
            alv_t = dpool.tile([P, NTAPS], fp32, name="alv_t") if PRECISE else None
            ident_t = dpool.tile([P, P], fp32, name="ident_t")
            stats = spool.tile([P, NTILES], fp32)
            m_loc = spool.tile([P, 1], fp32)
            m_row = spool.tile([1, P], fp32)
            pt = psumt_pool.tile([1, P], fp32, name="pt")
            inv_b = spool.tile([P, 1], fp32)
            cc_sb = spool.tile([1, 8], fp32)
            g_all = spool.tile([P, 8 * N_CORES], fp32)

            cc_in = dram_pool.tile([1, 8], fp32)
            cc_out = dram_pool.tile([N_CORES, 8], fp32, addr_space="Shared")

            # the first matmul (largest delay tap) needs only its own diag
            # slice and the first x chunk — issue those first
            first_tap = max(range(NTAPS), key=lambda n: DELAYS[n])
            fs0, fs1 = first_tap * P, (first_tap + 1) * P
            nhalf = dsets * NTAPS * P // 2
            nc.sync.dma_start(diag_t[:, fs0:fs1], diags.ap()[:, fs0:fs1])
            for i, (c0, c1) in enumerate(XCH):
                nc.sync.dma_start(xh_t[:, c0:c1], shard_ap(xh_d, c0, c1))
                if PRECISE:
                    nc.sync.dma_start(xl_t[:, c0:c1], shard_ap(xl_d, c0, c1))
                if i == 0:
                    if fs0 > 0:
                        nc.sync.dma_start(diag_t[:, 0:fs0], diags.ap()[:, 0:fs0])
                    if fs1 < nhalf:
                        nc.sync.dma_start(
                            diag_t[:, fs1:nhalf], diags.ap()[:, fs1:nhalf]
                        )
                    if dsets > 1:
                        nc.sync.dma_start(
                            diag_t[:, nhalf:], diags.ap()[:, nhalf:]
                        )
                    if PRECISE:
                        nc.sync.dma_start(alv_t[:], alv.ap())
                    nc.sync.dma_start(ident_t[:], ident.ap())

            # warm up the PE (HAM/pstate ramp) with junk matmuls into the
            # scratch psum bank while the input DMAs stream; pt is fully
            # overwritten later by the start=True transpose
            for _ in range(10):
                nc.tensor.matmul(
                    pt[:], diag_t[:, fs0 : fs0 + 1], diag_t[:, fs0 : fs0 + P],
                    start=True, stop=True,
                )

            # delay taps: diagonal matmuls accumulating in PSUM.
            # Descending delay order so the first matmuls of a tile only
            # need the earliest input columns (x chunks stream in behind).
            # The al*xh pass of the 4 shortest-delay taps runs on the DVE
            # instead (full-fp32 scalars) to rebalance PE (~73us) vs DVE
            # (~66us) busy time.
            order = sorted(range(NTAPS), key=lambda n: -DELAYS[n])

            def tile_plan(j):
                # the last tile keeps everything on the PE so its (serial)
                # DVE evac chain off the critical tail stays short
                dve_c = set(order[-4:]) if (PRECISE and j < NTILES - 1) else set()
                passes = []  # (diag_set, tap, x_tile)
                for n in order:
                    passes.append((0, n, xh_t))  # ah * xh
                    if PRECISE:
                        passes.append((0, n, xl_t))  # ah * xl
                        if n not in dve_c:
                            passes.append((1, n, xh_t))  # al * xh
                return passes, dve_c

            for j in range(NTILES):
                ps = psum_pool.tile([P, TILE], fp32, tag="ps", name=f"ps_{j}")
                base = D + j * TILE
                passes, dve_c_taps = tile_plan(j)
                for i, (s, n, xt) in enumerate(passes):
                    lhsT = diag_t[:, (s * NTAPS + n) * P : (s * NTAPS + n + 1) * P]
                    rhs = xt[:, base - DELAYS[n] : base - DELAYS[n] + TILE]
                    nc.tensor.matmul(
                        ps[:], lhsT, rhs,
                        start=(i == 0), stop=(i == len(passes) - 1),
                    )
                # evacuate PSUM -> SBUF adding the exact identity (x=xh+xl)
                ysl = y_tile[:, j * TILE : (j + 1) * TILE]
                nc.vector.scalar_tensor_tensor(
                    ysl, ps[:], 1.0, xh_t[:, base : base + TILE],
                    op0=mybir.AluOpType.mult, op1=mybir.AluOpType.add,
                )
                if PRECISE:
                    nc.vector.scalar_tensor_tensor(
                        ysl, ysl, 1.0, xl_t[:, base : base + TILE],
                        op0=mybir.AluOpType.mult, op1=mybir.AluOpType.add,
                    )
                    for n in dve_c_taps:
                        sh = slice(base - DELAYS[n], base - DELAYS[n] + TILE)
                        nc.vector.scalar_tensor_tensor(
                            ysl, xh_t[:, sh], alv_t[:, n : n + 1], ysl,
                            op0=mybir.AluOpType.mult, op1=mybir.AluOpType.add,
                        )
                nc.vector.tensor_reduce(
                    stats[:, j : j + 1], ysl,
                    axis=mybir.AxisListType.X, op=mybir.AluOpType.max,
                    apply_absolute_value=True,
                )

            # local max: tiles 0..14 are reduced+gathered+maxed early (all
            # hidden under the PE phase); the last tile's stats column is
            # partition-transposed on the (by then idle) PE and reduced into
            # a second slot of cc_sb -- the post-AG reduce maxes over both.
            nc.vector.memset(cc_sb[:], 0.0)
            nc.vector.tensor_reduce(
                m_loc[:, 0:1], stats[:, 0 : NTILES - 1],
                axis=mybir.AxisListType.X, op=mybir.AluOpType.max,
            )
            nc.sync.dma_start(m_row[0:1, :], m_loc[:, 0:1])
            nc.vector.tensor_reduce(
                cc_sb[0:1, 0:1], m_row[0:1, :],
                axis=mybir.AxisListType.X, op=mybir.AluOpType.max,
            )
            nc.tensor.transpose(pt[:], stats[:, NTILES - 1 : NTILES], ident_t[:])
            nc.vector.tensor_reduce(
                cc_sb[0:1, 1:2], pt[:],
                axis=mybir.AxisListType.X, op=mybir.AluOpType.max,
            )

            # global max across cores: AllGather the 8 local maxima
            nc.sync.dma_start(cc_in[:], cc_sb[:])
            nc.gpsimd.collective_compute(
                "AllGather",
                mybir.AluOpType.bypass,
                replica_groups=[list(range(N_CORES))],
                ins=[cc_in[:].opt()],
                outs=[cc_out[:].opt()],
            )
            # broadcast-read all 64 gathered floats into every partition
            nc.sync.dma_start(
                g_all[:],
                bass.AP(tensor=cc_out.tensor, offset=0, ap=[[0, P], [1, 8 * N_CORES]]),
            )
            nc.vector.tensor_reduce(
                inv_b[:], g_all[:], axis=mybir.AxisListType.X, op=mybir.AluOpType.max
            )
            nc.vector.reciprocal(inv_b[:], inv_b[:])

            # scale + store (DVE-heavy split; small first chunk so the first
            # output DMA starts as soon as possible after the collective)
            SCHUNKS = [
                ("v", 0, 256), ("v", 256, 1280), ("v", 1280, 2304),
                ("v", 2304, 3328), ("v", 3328, 4352), ("v", 4352, 5120),
                ("a", 5120, 6144), ("a", 6144, 7168), ("a", 7168, 8192),
            ]
            for eng, c0, c1 in SCHUNKS:
                ysl = y_tile[:, c0:c1]
                if eng == "v":
                    nc.vector.tensor_scalar_mul(ysl, ysl, inv_b[:, 0:1])
                else:
                    nc.scalar.mul(ysl, ysl, inv_b[:, 0:1])
                nc.sync.dma_start(shard_ap(out, c0, c1), ysl)

    nc.compile()
    return nc


def _prep_inputs(input_sig, feedback_gain, orthogonal_matrix):
    x = np.ascontiguousarray(np.asarray(input_sig, dtype=np.float32)).reshape(T)
    g = np.asarray(feedback_gain, dtype=np.float32)
    q = np.asarray(orthogonal_matrix, dtype=np.float32)
    coeff = (q.sum(axis=0) * g).astype(np.float32)  # [8]

    xpad = np.concatenate([np.zeros(D, np.float32), x])  # [D + T]
    idx = np.arange(P)

    if PRECISE:
        bf = ml_dtypes.bfloat16
        xh = xpad.astype(bf)
        xl = (xpad - xh.astype(np.float32)).astype(bf)
        ah = coeff.astype(bf)
        al_f32 = (coeff - ah.astype(np.float32)).astype(np.float32)
        al = al_f32.astype(bf)
        diags = np.zeros((P, 2 * NTAPS * P), dtype=bf)
        for n in range(NTAPS):
            diags[idx, n * P + idx] = ah[n]
            diags[idx, (NTAPS + n) * P + idx] = al[n]
        alv = np.tile(al_f32.reshape(1, NTAPS), (P, 1)).astype(np.float32)
    else:
        xh = xpad
        ah = coeff
        diags = np.zeros((P, NTAPS * P), dtype=np.float32)
        for n in range(NTAPS):
            diags[idx, n * P + idx] = ah[n]

    ident = np.eye(P, dtype=np.float32)
    in_maps = []
    for c in range(N_CORES):
        sl = slice(c * T_CORE, c * T_CORE + D + T_CORE)
        m = {
            "xh": np.ascontiguousarray(xh[sl]).reshape(1, D + T_CORE),
            "diags": diags,
        }
        m["ident"] = ident
        if PRECISE:
            m["xl"] = np.ascontiguousarray(xl[sl]).reshape(1, D + T_CORE)
            m["alv"] = alv
        in_maps.append(m)
    return in_maps


def _run(in_maps, trace=False):
    if "nc" not in _cache:
        _cache["nc"] = _build_nc()
    nc = _cache["nc"]
    res = run_bass_kernel_spmd(
        nc, in_maps, core_ids=list(range(N_CORES)), trace=trace
    )
    outs = [r["out"].reshape(T_CORE) for r in res.results]
    full = np.concatenate(outs).reshape(1, T)
    return full, res


def kernel(input_sig, feedback_gain, orthogonal_matrix):
    in_maps = _prep_inputs(input_sig, feedback_gain, orthogonal_matrix)
    try:
        full, _ = _run(in_maps, trace=False)
    except Exception:
        # one retry: a freshly-attached terminal occasionally reports a
        # transient device-unrecoverable error on the first execution
        full, _ = _run(in_maps, trace=False)
    return full



# revision 19
# speedup vs baseline: 1.8992x; 1.8992x over previous
"""FDN reverb kernel for 8x TRN2 NeuronCores.

Computes out = y / max|y| with y[t] = x[t] + sum_n a_n * x[t - d_n],
where a_n = (sum_j Q[j, n]) * g[n]  (the MIX=0.5 factor cancels in the
normalization).

Sharding: time axis split into 8 contiguous shards of 1M samples; each
core's input carries a max-delay halo from the previous shard (zeros for
core 0).  On-core layout is partition-major: partition p holds samples
[p*F, p*F + F) of the shard plus a D-sample halo in front, so every
delayed read is a free-axis offset.

All data is fp16 (measured end-to-end rel err ~1.5e-3 vs the fp32
reference).  Per 1024-col chunk the 8 delay taps are split across
engines: 6 taps run on the tensor engine as diagonal-stationary matmuls
accumulating in PSUM, which the scalar engine evacuates to fp16 y; 2
taps + the identity run on the DVE as tensor_scalar (4x mode) +
tensor_tensor (2x mode) pairs into y_v, which a gpsimd-initiated
SBUF->SBUF DMA accumulates into y (accum_op=add, CCE inline ALU); the
DVE folds a running abs-max.  A tiny AllGather(max) yields the global
normalizer; DVE/ACT scale; DMA out.
"""

import numpy as np

import concourse.bacc as bacc
import concourse.bass as bass
import concourse.mybir as mybir
import concourse.tile as tile
from concourse.bass_utils import run_bass_kernel_spmd

# ---- problem constants (hardcoded; must match the reference) ----
SAMPLE_RATE = 48000
DELAYS_SEC = [0.0297, 0.0371, 0.0411, 0.0437, 0.0533, 0.0617, 0.0731, 0.0797]
DELAYS = [int(d * SAMPLE_RATE) for d in DELAYS_SEC]  # [1425,...,3825]
NTAPS = len(DELAYS)  # 8
T = 8388608
N_CORES = 8
T_CORE = T // N_CORES  # 1048576
P = 128
F = T_CORE // P  # 8192 samples per partition row
D = 3840  # halo (>= max delay 3825), 128-aligned
CH = 1024  # processing chunk (free dim)
NCH = F // CH  # 8
HT = 512  # PSUM bank tile / matmul moving width

# tap split: big delays on PE (early columns -> PE starts first),
# the two smallest + identity on the DVE; the final chunk runs all-PE
PE_DELAYS = [3825, 3508, 2961, 2558, 2097, 1972]
DVE_DELAYS = [1780, 1425]
MERGE_DMA = True  # merge y_v into y via gpsimd accum-DMA (else PE matmul)
NWARM = 8  # PE p-state warmup matmuls

# in-DMA column chunks of the [128, D+F] overlapped row view
XBOUNDS = [0, 640] + [640 + 1424 * (k + 1) for k in range(8)]  # ... 12032

# out-DMA / scale chunks (first/last small so the tail pipeline starts fast)
SBOUNDS = [0, 512] + [512 + 1024 * (k + 1) for k in range(7)] + [8192]

_cache = {}


def _build_nc():
    fp32 = mybir.dt.float32
    fp16 = mybir.dt.float16
    nblk = len(PE_DELAYS) + 3  # 6 diagonals + identity + 2 small-tap diagonals

    nc = bacc.Bacc(
        "TRN2",
        target_bir_lowering=False,
        debug=False,
        enable_asserts=False,
        num_devices=N_CORES,
    )

    xh_d = nc.dram_tensor("xh", [1, D + T_CORE], fp16, kind="ExternalInput")
    # stationaries: tap diagonals (+ identity for the merge if on PE), fp16
    wmat = nc.dram_tensor("wmat", [P, nblk * P], fp16, kind="ExternalInput")
    # full-precision per-partition coeff scalars for the DVE taps
    avec = nc.dram_tensor("avec", [P, 8], fp32, kind="ExternalInput")
    out = nc.dram_tensor("out", [1, T_CORE], fp16, kind="ExternalOutput")

    def shard_ap(t, c0, c1):
        # columns [c0, c1) of the overlapped [128, D+F] row view
        return bass.AP(tensor=t, offset=c0, ap=[[F, P], [1, c1 - c0]])

    with tile.TileContext(nc) as tc:
        with (
            tc.tile_pool(name="xpool", bufs=1) as xpool,
            tc.tile_pool(name="ypool", bufs=1) as ypool,
            tc.tile_pool(name="dpool", bufs=1) as dpool,
            tc.tile_pool(name="vpool", bufs=4) as vpool,
            tc.tile_pool(name="tpool", bufs=4) as tpool,
            tc.tile_pool(name="spool", bufs=1) as spool,
            tc.tile_pool(name="psum", bufs=7, space="PSUM") as psum_pool,
            tc.tile_pool(name="psumt", bufs=1, space="PSUM") as psumt_pool,
            tc.tile_pool(name="dram", bufs=1, space="DRAM") as dram_pool,
        ):
            x_t = xpool.tile([P, D + F], fp16)
            y_t = ypool.tile([P, F], fp16)
            w_t = dpool.tile([P, nblk * P], fp16)
            av_t = dpool.tile([P, 8], fp32, name="av_t")
            st = spool.tile([P, 16], fp32, name="st")
            m_loc = spool.tile([P, 1], fp32)
            cc_sb = spool.tile([1, 8], fp32)
            g_all = spool.tile([P, 8 * N_CORES], fp32)
            inv_b = spool.tile([P, 1], fp32)

            cc_in = dram_pool.tile([1, 8], fp32)
            cc_out = dram_pool.tile([N_CORES, 8], fp32, addr_space="Shared")

            jtile = dpool.tile([P, P], fp16, name="jtile")
            nc.vector.memset(jtile[:], 1.0)
            # PE p-state warmup: junk matmuls on the memset tile (no DMA dep)
            pwarm = psumt_pool.tile([P, P], fp32, tag="pt", name="pwarm")
            for _ in range(NWARM):
                nc.tensor.matmul(
                    pwarm[:], jtile[:], jtile[:], start=True, stop=True,
                )

            nc.sync.dma_start(w_t[:], wmat.ap())
            for i, (c0, c1) in enumerate(zip(XBOUNDS[:-1], XBOUNDS[1:])):
                nc.sync.dma_start(x_t[:, c0:c1], shard_ap(xh_d, c0, c1))
                if i == 1:
                    nc.sync.dma_start(av_t[:], avec.ap())

            nc.vector.memset(cc_sb[:], 0.0)

            d0, d1 = DVE_DELAYS

            def emit_yv(j):
                # DVE: 2 taps via tensor_scalar (4x) + tensor_tensor (2x),
                # identity folded into the first add's second operand
                b = D + j * CH
                yv = vpool.tile([P, CH], fp16, tag="yv", name=f"yv_{j}")
                t0 = tpool.tile([P, CH], fp16, tag="ts", name=f"t0_{j}")
                nc.vector.tensor_scalar_mul(
                    t0[:], x_t[:, b - d0 : b - d0 + CH], av_t[:, 0:1]
                )
                nc.vector.tensor_tensor(
                    yv[:], t0[:], x_t[:, b : b + CH], op=mybir.AluOpType.add
                )
                t1 = tpool.tile([P, CH], fp16, tag="ts", name=f"t1_{j}")
                nc.vector.tensor_scalar_mul(
                    t1[:], x_t[:, b - d1 : b - d1 + CH], av_t[:, 1:2]
                )
                nc.vector.tensor_tensor(
                    yv[:], yv[:], t1[:], op=mybir.AluOpType.add
                )
                return yv

            yv_q = [emit_yv(0), emit_yv(1)]
            for j in range(NCH):
                base = D + j * CH
                c0 = j * CH
                pe_merge = j >= NCH - 2  # last two chunks merge on the PE
                yv = yv_q.pop(0)

                for h in range(2):
                    hb = base + h * HT
                    ps = psum_pool.tile([P, HT], fp32, tag="ps", name=f"ps_{j}_{h}")
                    for t_i, dd in enumerate(PE_DELAYS):
                        nc.tensor.matmul(
                            ps[:],
                            w_t[:, t_i * P : (t_i + 1) * P],
                            x_t[:, hb - dd : hb - dd + HT],
                            start=(t_i == 0),
                            stop=(not pe_merge)
                            and (t_i == len(PE_DELAYS) - 1),
                        )
                    if pe_merge:
                        nc.tensor.matmul(
                            ps[:],
                            w_t[:, 6 * P : 7 * P],
                            yv[:, h * HT : (h + 1) * HT],
                            start=False, stop=True,
                        )
                    nc.scalar.copy(y_t[:, c0 + h * HT : c0 + (h + 1) * HT], ps[:])

                if j + 2 < NCH:
                    yv_q.append(emit_yv(j + 2))
                if not pe_merge:
                    nc.gpsimd.dma_start(
                        y_t[:, c0 : c0 + CH], yv[:],
                        accum_op=mybir.AluOpType.add,
                    )
                # per-chunk |max| stats on the DVE (gpsimd's cross-partition
                # reduce ignores apply_absolute_value, so abs happens here);
                # the final chunk folds as two halves in the drain
                if j < NCH - 1:
                    nc.vector.tensor_reduce(
                        st[:, j : j + 1], y_t[:, c0 : c0 + CH],
                        axis=mybir.AxisListType.X, op=mybir.AluOpType.max,
                        apply_absolute_value=True,
                    )

            # local max: last chunk via two absolute half-reduces, then one
            # tiny reduce over stats columns and a gpsimd cross-partition fold
            # (values are non-negative by then)
            lb = (NCH - 1) * CH
            nc.vector.tensor_reduce(
                st[:, NCH - 1 : NCH], y_t[:, lb : lb + HT],
                axis=mybir.AxisListType.X, op=mybir.AluOpType.max,
                apply_absolute_value=True,
            )
            nc.vector.tensor_reduce(
                st[:, NCH : NCH + 1], y_t[:, lb + HT : lb + CH],
                axis=mybir.AxisListType.X, op=mybir.AluOpType.max,
                apply_absolute_value=True,
            )
            nc.vector.tensor_reduce(
                m_loc[:, 0:1], st[:, 0 : NCH + 1],
                axis=mybir.AxisListType.X, op=mybir.AluOpType.max,
            )
            nc.gpsimd.tensor_reduce(
                cc_sb[0:1, 0:1], m_loc[:, 0:1], axis=mybir.AxisListType.XYZWC,
                op=mybir.AluOpType.max,
            )

            # global max across cores
            nc.sync.dma_start(cc_in[:], cc_sb[:])
            nc.gpsimd.collective_compute(
                "AllGather",
                mybir.AluOpType.bypass,
                replica_groups=[list(range(N_CORES))],
                ins=[cc_in[:].opt()],
                outs=[cc_out[:].opt()],
            )
            nc.sync.dma_start(
                g_all[:],
                bass.AP(tensor=cc_out.tensor, offset=0, ap=[[0, P], [1, 8 * N_CORES]]),
            )
            nc.vector.tensor_reduce(
                inv_b[:], g_all[:], axis=mybir.AxisListType.X, op=mybir.AluOpType.max
            )
            nc.vector.reciprocal(inv_b[:], inv_b[:])

            # scale + store, DVE/ACT alternating, DMA out per chunk
            for i, (c0, c1) in enumerate(zip(SBOUNDS[:-1], SBOUNDS[1:])):
                ysl = y_t[:, c0:c1]
                if i % 2 == 0:
                    nc.vector.tensor_scalar_mul(ysl, ysl, inv_b[:, 0:1])
                else:
                    nc.scalar.mul(ysl, ysl, inv_b[:, 0:1])
                nc.sync.dma_start(shard_ap(out, c0, c1), ysl)

    nc.compile()
    return nc


def _prep_inputs(input_sig, feedback_gain, orthogonal_matrix):
    x = np.ascontiguousarray(np.asarray(input_sig, dtype=np.float32)).reshape(T)
    g = np.asarray(feedback_gain, dtype=np.float32)
    q = np.asarray(orthogonal_matrix, dtype=np.float32)
    coeff = (q.sum(axis=0) * g).astype(np.float32)  # [8]
    di = {dd: i for i, dd in enumerate(DELAYS)}
    nblk = len(PE_DELAYS) + 3

    xpad = np.concatenate([np.zeros(D, np.float32), x]).astype(np.float16)
    idx = np.arange(P)

    wmat = np.zeros((P, nblk * P), dtype=np.float16)
    for t_i, dd in enumerate(PE_DELAYS):
        wmat[idx, t_i * P + idx] = coeff[di[dd]].astype(np.float16)
    wmat[idx, 6 * P + idx] = np.float16(1.0)
    for bi, dd in ((7, DVE_DELAYS[0]), (8, DVE_DELAYS[1])):
        wmat[idx, bi * P + idx] = coeff[di[dd]].astype(np.float16)

    avec = np.zeros((P, 8), dtype=np.float32)
    for i, dd in enumerate(DVE_DELAYS):
        avec[:, i] = coeff[di[dd]]

    in_maps = []
    for c in range(N_CORES):
        sl = slice(c * T_CORE, c * T_CORE + D + T_CORE)
        in_maps.append({
            "xh": np.ascontiguousarray(xpad[sl]).reshape(1, D + T_CORE),
            "wmat": wmat,
            "avec": avec,
        })
    return in_maps


def _run(in_maps, trace=False):
    if "nc" not in _cache:
        _cache["nc"] = _build_nc()
    nc = _cache["nc"]
    res = run_bass_kernel_spmd(
        nc, in_maps, core_ids=list(range(N_CORES)), trace=trace
    )
    outs = [r["out"].reshape(T_CORE).astype(np.float32) for r in res.results]
    full = np.concatenate(outs).reshape(1, T)
    return full, res


def kernel(input_sig, feedback_gain, orthogonal_matrix):
    in_maps = _prep_inputs(input_sig, feedback_gain, orthogonal_matrix)
    try:
        full, _ = _run(in_maps, trace=False)
    except Exception:
        # one retry: a freshly-attached terminal occasionally reports a
        # transient device-unrecoverable error on the first execution
        full, _ = _run(in_maps, trace=False)
    return full


# revision 25
# speedup vs baseline: 1.9599x; 1.0319x over previous
"""FDN reverb kernel for 8x TRN2 NeuronCores.

Computes out = y / max|y| with y[t] = x[t] + sum_n a_n * x[t - d_n],
where a_n = (sum_j Q[j, n]) * g[n]  (the MIX=0.5 factor cancels in the
normalization).

Sharding: time axis split into 8 contiguous shards of 1M samples; each
core's input carries a max-delay halo from the previous shard (zeros for
core 0).  On-core layout is partition-major: partition p holds samples
[p*F, p*F + F) of the shard plus a D-sample halo in front, so every
delayed read is a free-axis offset.

All data is fp16 (measured end-to-end rel err ~1.5e-3 vs the fp32
reference).  Per 1024-col chunk the 8 delay taps are split across
engines: 6 taps run on the tensor engine as diagonal-stationary matmuls
accumulating in PSUM, which the scalar engine evacuates to fp16 y; 2
taps + the identity run on the DVE as tensor_scalar (4x mode) +
tensor_tensor (2x mode) pairs into y_v, which a gpsimd-initiated
SBUF->SBUF DMA accumulates into y (accum_op=add, CCE inline ALU); the
DVE folds a running abs-max.  A tiny AllGather(max) yields the global
normalizer; DVE/ACT scale; DMA out.
"""

import numpy as np

import concourse.bacc as bacc
import concourse.bass as bass
import concourse.mybir as mybir
import concourse.tile as tile
from concourse.bass_utils import run_bass_kernel_spmd

# ---- problem constants (hardcoded; must match the reference) ----
SAMPLE_RATE = 48000
DELAYS_SEC = [0.0297, 0.0371, 0.0411, 0.0437, 0.0533, 0.0617, 0.0731, 0.0797]
DELAYS = [int(d * SAMPLE_RATE) for d in DELAYS_SEC]  # [1425,...,3825]
NTAPS = len(DELAYS)  # 8
T = 8388608
N_CORES = 8
T_CORE = T // N_CORES  # 1048576
P = 128
F = T_CORE // P  # 8192 samples per partition row
D = 3840  # halo (>= max delay 3825), 128-aligned
CH = 1024  # processing chunk (free dim)
NCH = F // CH  # 8
HT = 512  # PSUM bank tile / matmul moving width

# tap split: big delays on PE (early columns -> PE starts first),
# the two smallest + identity on the DVE; the final chunk runs all-PE
PE_DELAYS = [3825, 3508, 2961, 2558, 2097, 1972]
DVE_DELAYS = [1780, 1425]
MERGE_DMA = True  # merge y_v into y via gpsimd accum-DMA (else PE matmul)
NWARM = 8  # PE p-state warmup matmuls

# in-DMA column chunks of the [128, D+F] overlapped row view
XBOUNDS = [0, 640] + [640 + 1424 * (k + 1) for k in range(8)]  # ... 12032

# out-DMA / scale chunks (first/last small so the tail pipeline starts fast)
SBOUNDS = [0, 512] + [512 + 1024 * (k + 1) for k in range(7)] + [8192]

_cache = {}


def _build_nc():
    fp32 = mybir.dt.float32
    fp16 = mybir.dt.float16
    u16 = mybir.dt.uint16
    nblk = len(PE_DELAYS) + 3  # 6 diagonals + identity + 2 small-tap diagonals

    nc = bacc.Bacc(
        "TRN2",
        target_bir_lowering=False,
        debug=False,
        enable_asserts=False,
        num_devices=N_CORES,
    )

    xh_d = nc.dram_tensor("xh", [1, D + T_CORE], fp16, kind="ExternalInput")
    # stationaries: tap diagonals (+ identity for the merge if on PE), fp16
    wmat = nc.dram_tensor("wmat", [P, nblk * P], fp16, kind="ExternalInput")
    # full-precision per-partition coeff scalars for the DVE taps
    avec = nc.dram_tensor("avec", [P, 8], fp32, kind="ExternalInput")
    out = nc.dram_tensor("out", [1, T_CORE], fp16, kind="ExternalOutput")

    def shard_ap(t, c0, c1):
        # columns [c0, c1) of the overlapped [128, D+F] row view
        return bass.AP(tensor=t, offset=c0, ap=[[F, P], [1, c1 - c0]])

    with tile.TileContext(nc) as tc:
        with (
            tc.tile_pool(name="xpool", bufs=1) as xpool,
            tc.tile_pool(name="ypool", bufs=1) as ypool,
            tc.tile_pool(name="dpool", bufs=1) as dpool,
            tc.tile_pool(name="vpool", bufs=4) as vpool,
            tc.tile_pool(name="tpool", bufs=4) as tpool,
            tc.tile_pool(name="spool", bufs=1) as spool,
            tc.tile_pool(name="psum", bufs=7, space="PSUM") as psum_pool,
            tc.tile_pool(name="psumt", bufs=1, space="PSUM") as psumt_pool,
            tc.tile_pool(name="dram", bufs=1, space="DRAM") as dram_pool,
        ):
            x_t = xpool.tile([P, D + F], fp16)
            y_t = ypool.tile([P, F], fp16)
            w_t = dpool.tile([P, nblk * P], fp16)
            av_t = dpool.tile([P, 8], fp32, name="av_t")
            st = spool.tile([P, 16], fp32, name="st")
            stu = spool.tile([1, 8], u16, name="stu")
            su1 = spool.tile([1, 1], u16, name="su1")
            m_loc = spool.tile([P, 1], fp32)
            cc_sb = spool.tile([1, 8], fp32)
            g_all = spool.tile([P, 8 * N_CORES], fp32)
            inv_b = spool.tile([P, 1], fp32)

            cc_in = dram_pool.tile([1, 8], fp32)
            cc_out = dram_pool.tile([N_CORES, 8], fp32, addr_space="Shared")

            jtile = dpool.tile([P, P], fp16, name="jtile")
            nc.vector.memset(jtile[:], 1.0)
            # PE p-state warmup: junk matmuls on the memset tile (no DMA dep)
            pwarm = psumt_pool.tile([P, P], fp32, tag="pt", name="pwarm")
            for _ in range(NWARM):
                nc.tensor.matmul(
                    pwarm[:], jtile[:], jtile[:], start=True, stop=True,
                )

            nc.sync.dma_start(w_t[:], wmat.ap())
            for i, (c0, c1) in enumerate(zip(XBOUNDS[:-1], XBOUNDS[1:])):
                nc.sync.dma_start(x_t[:, c0:c1], shard_ap(xh_d, c0, c1))
                if i == 1:
                    nc.sync.dma_start(av_t[:], avec.ap())

            nc.vector.memset(cc_sb[:], 0.0)
            nc.vector.memset(st[:], 0.0)

            d0, d1 = DVE_DELAYS

            def emit_yv(j):
                # DVE: 2 taps via tensor_scalar (4x) + tensor_tensor (2x),
                # identity folded into the first add's second operand
                b = D + j * CH
                yv = vpool.tile([P, CH], fp16, tag="yv", name=f"yv_{j}")
                t0 = tpool.tile([P, CH], fp16, tag="ts", name=f"t0_{j}")
                nc.vector.tensor_scalar_mul(
                    t0[:], x_t[:, b - d0 : b - d0 + CH], av_t[:, 0:1]
                )
                nc.vector.tensor_tensor(
                    yv[:], t0[:], x_t[:, b : b + CH], op=mybir.AluOpType.add
                )
                t1 = tpool.tile([P, CH], fp16, tag="ts", name=f"t1_{j}")
                nc.vector.tensor_scalar_mul(
                    t1[:], x_t[:, b - d1 : b - d1 + CH], av_t[:, 1:2]
                )
                nc.vector.tensor_tensor(
                    yv[:], yv[:], t1[:], op=mybir.AluOpType.add
                )
                return yv

            yv_q = [emit_yv(0), emit_yv(1)]
            for j in range(NCH):
                base = D + j * CH
                c0 = j * CH
                pe_merge = j >= NCH - 2  # last two chunks merge on the PE
                last = j == NCH - 1
                yv = yv_q.pop(0)

                # the final chunk tapers 512/256/256 so the drain pipeline
                # (evac -> abs-reduce) gets short tail pieces
                widths = [HT, HT // 2, HT // 2] if last else [HT, HT]
                off = 0
                for h, wd in enumerate(widths):
                    hb = base + off
                    ps = psum_pool.tile([P, HT], fp32, tag="ps", name=f"ps_{j}_{h}")
                    for t_i, dd in enumerate(PE_DELAYS):
                        nc.tensor.matmul(
                            ps[:, :wd],
                            w_t[:, t_i * P : (t_i + 1) * P],
                            x_t[:, hb - dd : hb - dd + wd],
                            start=(t_i == 0),
                            stop=(not pe_merge)
                            and (t_i == len(PE_DELAYS) - 1),
                        )
                    if pe_merge:
                        nc.tensor.matmul(
                            ps[:, :wd],
                            w_t[:, 6 * P : 7 * P],
                            yv[:, off : off + wd],
                            start=False, stop=True,
                        )
                    nc.scalar.copy(y_t[:, c0 + off : c0 + off + wd], ps[:, :wd])
                    if last:
                        nc.vector.tensor_reduce(
                            st[:, NCH - 1 + h : NCH + h],
                            y_t[:, c0 + off : c0 + off + wd],
                            axis=mybir.AxisListType.X, op=mybir.AluOpType.max,
                            apply_absolute_value=True,
                        )
                    off += wd

                if j + 2 < NCH:
                    yv_q.append(emit_yv(j + 2))
                if not pe_merge:
                    nc.gpsimd.dma_start(
                        y_t[:, c0 : c0 + CH], yv[:],
                        accum_op=mybir.AluOpType.add,
                    )
                # per-chunk |max| stats: odd dma-merged chunks compute |y|
                # cheaply on the DVE (bitwise-and on the u16 view, 4x mode)
                # and fold on gpsimd as a u16 cross-partition max; the rest
                # reduce with absolute on the DVE; final chunk in the drain
                if j in (1, 3, 5):
                    ay = tpool.tile([P, CH], fp16, tag="ay", name=f"ay_{j}")
                    nc.vector.tensor_scalar(
                        ay[:].bitcast(u16), y_t[:, c0 : c0 + CH].bitcast(u16),
                        0x7FFF, None, op0=mybir.AluOpType.bitwise_and,
                    )
                    nc.gpsimd.tensor_reduce(
                        stu[0:1, (j - 1) // 2 : (j + 1) // 2], ay[:].bitcast(u16),
                        axis=mybir.AxisListType.XYZWC, op=mybir.AluOpType.max,
                    )
                    if j == 5:
                        nc.gpsimd.tensor_reduce(
                            su1[0:1, 0:1], stu[0:1, 0:3],
                            axis=mybir.AxisListType.XYZWC,
                            op=mybir.AluOpType.max,
                        )
                elif j < NCH - 1:
                    nc.vector.tensor_reduce(
                        st[:, j : j + 1], y_t[:, c0 : c0 + CH],
                        axis=mybir.AxisListType.X, op=mybir.AluOpType.max,
                        apply_absolute_value=True,
                    )

            # local max: fold stats columns, gpsimd partition fold; the u16
            # row was folded early (hidden under the stream)
            nc.vector.tensor_reduce(
                m_loc[:, 0:1], st[:, 0 : NCH + 2],
                axis=mybir.AxisListType.X, op=mybir.AluOpType.max,
            )
            nc.gpsimd.tensor_reduce(
                cc_sb[0:1, 0:1], m_loc[:, 0:1], axis=mybir.AxisListType.XYZWC,
                op=mybir.AluOpType.max,
            )
            nc.vector.tensor_copy(
                cc_sb[0:1, 1:2], su1[0:1, 0:1].bitcast(fp16)
            )
            # global max across cores
            nc.sync.dma_start(cc_in[:], cc_sb[:])
            nc.gpsimd.collective_compute(
                "AllGather",
                mybir.AluOpType.bypass,
                replica_groups=[list(range(N_CORES))],
                ins=[cc_in[:].opt()],
                outs=[cc_out[:].opt()],
            )
            nc.sync.dma_start(
                g_all[:],
                bass.AP(tensor=cc_out.tensor, offset=0, ap=[[0, P], [1, 8 * N_CORES]]),
            )
            nc.vector.tensor_reduce(
                inv_b[:], g_all[:], axis=mybir.AxisListType.X, op=mybir.AluOpType.max
            )
            nc.vector.reciprocal(inv_b[:], inv_b[:])

            # scale + store, DVE/ACT alternating, DMA out per chunk
            for i, (c0, c1) in enumerate(zip(SBOUNDS[:-1], SBOUNDS[1:])):
                ysl = y_t[:, c0:c1]
                if i % 2 == 0:
                    nc.vector.tensor_scalar_mul(ysl, ysl, inv_b[:, 0:1])
                else:
                    nc.scalar.mul(ysl, ysl, inv_b[:, 0:1])
                nc.sync.dma_start(shard_ap(out, c0, c1), ysl)

    nc.compile()
    return nc


def _prep_inputs(input_sig, feedback_gain, orthogonal_matrix):
    x = np.ascontiguousarray(np.asarray(input_sig, dtype=np.float32)).reshape(T)
    g = np.asarray(feedback_gain, dtype=np.float32)
    q = np.asarray(orthogonal_matrix, dtype=np.float32)
    coeff = (q.sum(axis=0) * g).astype(np.float32)  # [8]
    di = {dd: i for i, dd in enumerate(DELAYS)}
    nblk = len(PE_DELAYS) + 3

    xpad = np.concatenate([np.zeros(D, np.float32), x]).astype(np.float16)
    idx = np.arange(P)

    wmat = np.zeros((P, nblk * P), dtype=np.float16)
    for t_i, dd in enumerate(PE_DELAYS):
        wmat[idx, t_i * P + idx] = coeff[di[dd]].astype(np.float16)
    wmat[idx, 6 * P + idx] = np.float16(1.0)
    for bi, dd in ((7, DVE_DELAYS[0]), (8, DVE_DELAYS[1])):
        wmat[idx, bi * P + idx] = coeff[di[dd]].astype(np.float16)

    avec = np.zeros((P, 8), dtype=np.float32)
    for i, dd in enumerate(DVE_DELAYS):
        avec[:, i] = coeff[di[dd]]

    in_maps = []
    for c in range(N_CORES):
        sl = slice(c * T_CORE, c * T_CORE + D + T_CORE)
        in_maps.append({
            "xh": np.ascontiguousarray(xpad[sl]).reshape(1, D + T_CORE),
            "wmat": wmat,
            "avec": avec,
        })
    return in_maps


def _run(in_maps, trace=False):
    if "nc" not in _cache:
        _cache["nc"] = _build_nc()
    nc = _cache["nc"]
    res = run_bass_kernel_spmd(
        nc, in_maps, core_ids=list(range(N_CORES)), trace=trace
    )
    outs = [r["out"].reshape(T_CORE).astype(np.float32) for r in res.results]
    full = np.concatenate(outs).reshape(1, T)
    return full, res


def kernel(input_sig, feedback_gain, orthogonal_matrix):
    in_maps = _prep_inputs(input_sig, feedback_gain, orthogonal_matrix)
    try:
        full, _ = _run(in_maps, trace=False)
    except Exception:
        # one retry: a freshly-attached terminal occasionally reports a
        # transient device-unrecoverable error on the first execution
        full, _ = _run(in_maps, trace=False)
    return full
